# revision 58
# baseline (speedup 1.0000x reference)
"""Trainium2 Bass kernel for nn_EntropyController (retrieval_knn).

Math (reference, for x = features.reshape(N, D)):
    d2_ij   = ||x_i - x_j||^2
    k_ij    = exp(-d2_ij / (2 t^2))
    p_ij    = k_ij / S_i,  S_i = sum_j k_ij
    H_i     = -sum_j p_ij log(p_ij + 1e-6)
    control = sigmoid(-(H - target)/t);  out = features * control[..., None]

Kernel strategy (block-row data parallel over 8 cores, M = N/8 rows each):
    p_ij (and therefore H and the outputs) is invariant under any per-row
    shift of the exponent, so instead of -d2/(2t^2) the device evaluates
        e_ij = (dot(x_i, x_j) - sq_i) / t^2
    (shift by sq_j/2 - sq_i/2 per row): the -sq_i bias is a per-partition
    activation bias, which makes the whole exponent computable by a plain
    matmul -- no augmentation row for the column term.  For gaussian-like
    data e_ij <= ~12 off-diagonal (no f32 overflow) and e_ii = 0.

    The N x N map is never materialized.  Per [128, C] chunk:
      - TensorE: m = dot(x_i, x_j) into PSUM (two K=128 matmuls).
      - ScalarE: k = exp(scale*m + bias_i) PSUM->SBUF, with accum_out
        producing the row-sum S for free.
      - VectorE: one fused scalar_tensor_tensor gives U = sum_j k*m.
    Then sum_j k*e = scale*U + bias*S, so
        H = log S - (scale*U + bias*S)/S.
    The reference's +1e-6 inside the log shifts H by log1p(1e-6) per
    dominant mode; -log1p(1e-6) is folded into the sigmoid bias.
    sigmoid is computed as 1/(1+exp(z)) to stay in the exp/ln ACT table set.
"""

import math
from contextlib import ExitStack

import numpy as np

N_CORES = 8
D = 256
P = 128
MM_N = 512  # moving free dim per matmul (one PSUM bank of f32)


def _legalize_waits(nc, limit=1):
    """The deployed walrus accepts at most one sync-wait command per
    instruction; Tile's scheduler freely emits more.  Hoist the excess onto
    injected same-engine EventSemaphore instructions placed directly before
    the offender (engine programs are sequential, so the waits still gate
    it)."""
    import bass_rust
    from concourse import mybir

    n_new = 0
    for fn in nc.m.functions:
        for blk in fn.blocks:
            out = []
            for inst in blk.instructions:
                si = getattr(inst, "sync_info", None)
                waits = list(si.on_wait) if (si is not None and si.on_wait) else []
                if len(waits) > limit:
                    excess, keep = waits[:-limit], waits[-limit:]
                    for i, w in enumerate(excess):
                        ev = bass_rust.InstEventSemaphore(
                            name=f"{inst.name}-waitshim-{i}",
                            engine=inst.engine,
                            ins=[],
                            outs=[],
                        )
                        ev.sync_info = mybir.SyncInfo(on_wait=[w], on_update=[])
                        nc.inst_map[ev.name] = ev
                        out.append(ev)
                        n_new += 1
                    inst.sync_info = mybir.SyncInfo(
                        on_wait=keep, on_update=list(si.on_update)
                    )
                out.append(inst)
            blk.instructions = out
    return n_new


# Test-harness hooks (the grading path leaves these alone).
TRACE = False
LAST_RESULT = None


def build_program(n_pts, n_rows, temperature, target_entropy, col_chunk=1024):
    import concourse.bass as bass
    import concourse.tile as tile
    from concourse import mybir

    f32 = mybir.dt.float32
    bf16 = mybir.dt.bfloat16
    AF = mybir.ActivationFunctionType
    ALU = mybir.AluOpType
    AX = mybir.AxisListType

    assert n_pts % col_chunk == 0 and col_chunk % MM_N == 0
    assert n_rows % P == 0 and n_pts % P == 0
    n_ptiles = n_pts // P
    n_rtiles = n_rows // P
    n_chunks = n_pts // col_chunk
    n_groups = col_chunk // MM_N

    t = float(temperature)
    scale0 = 1.0 / (t * t)
    # z = (H - target - log1p(1e-6)) / t ; control = 1/(1+exp(z))
    sig_scale = 1.0 / t
    sig_bias = -(float(target_entropy) + math.log1p(1e-6)) / t

    nc = bass.Bass()
    x_full = nc.declare_dram_parameter("x_full", [n_pts, D], f32, isOutput=False)
    ident_in = nc.declare_dram_parameter("ident_in", [P, P], f32, isOutput=False)
    x_rows = nc.declare_dram_parameter("x_rows", [n_rows, D], f32, isOutput=False)
    out_feat = nc.declare_dram_parameter("out_feat", [n_rows, D], f32, isOutput=True)
    out_ctrl = nc.declare_dram_parameter("out_ctrl", [n_rows], f32, isOutput=True)

    xf = x_full[:, :].rearrange("(t p) d -> t p d", p=P)
    xr = x_rows[:, :].rearrange("(t p) d -> t p d", p=P)
    of_re = out_feat[:, :].rearrange("(t p) d -> t p d", p=P)

    with tile.TileContext(nc) as tc, ExitStack() as ctx:
        persist = ctx.enter_context(tc.tile_pool(name="persist", bufs=1))
        # 3x [128,1024] f32 accumulators (6 banks) + 2x [128,512] transpose
        # tiles (2 banks) = all 8 PSUM banks
        psum_m = ctx.enter_context(tc.tile_pool(name="psum_m", bufs=3, space="PSUM"))
        psum_t = ctx.enter_context(tc.tile_pool(name="psum_t", bufs=2, space="PSUM"))

        xT0 = persist.tile([P, n_pts], bf16)  # dims 0..127, all points
        xT1 = persist.tile([P, n_pts], bf16)  # dims 128..255
        lhsT0 = persist.tile([P, n_rows], bf16)
        lhsT1 = persist.tile([P, n_rows], bf16)
        xrn = persist.tile([P, n_rtiles, D], f32)  # this core's rows, natural
        sqr_all = persist.tile([P, n_rtiles], f32)  # per-row |x|^2 (f32)
        bias_all = persist.tile([P, n_rtiles], f32)  # -sq_i/t^2
        ctrl_all = persist.tile([P, n_rtiles], f32)
        S_all = [persist.tile([P, n_chunks], f32, name=f"S_acc_{r}", tag=f"S_acc_{r}")
                 for r in range(n_rtiles)]
        U_all = [persist.tile([P, n_chunks], f32, name=f"U_acc_{r}", tag=f"U_acc_{r}")
                 for r in range(n_rtiles)]
        zero_b = persist.tile([P, 1], f32)
        sig_b = persist.tile([P, 1], f32)
        ident = persist.tile([P, P], f32)  # for PE-mode transposes (f32 in/out)
        XN = persist.tile([P, n_ptiles, D], f32)  # full point set, natural

        sigb_all = persist.tile([P, n_rtiles], f32)  # sig_b - sig_scale*bias_i

        nc.vector.memset(zero_b, 0.0)
        nc.vector.memset(sig_b, sig_bias)
        # identity comes from DRAM: removes the gpsimd memset+affine_select
        # from the transpose critical path
        nc.sync.dma_start(out=ident, in_=ident_in[:, :])

        # PE-mode f32 128x128 transposes, 4 tiles batched per PSUM bank tile;
        # the ACT PSUM->SBUF copy performs the f32->bf16 cast.  Transposes get
        # their own PSUM tag so they pipeline independently of the matmul
        # accumulators.
        TG = 4  # point-tiles per transpose group

        copy_rr = [0]  # round-robin PSUM->SBUF copies: 2 on ACT, 1 on DVE

        def transpose_group(src_tile, grp, qn, dst0, dst1):
            ps0 = psum_t.tile([P, TG * P], f32, tag="tpsum")
            ps1 = psum_t.tile([P, TG * P], f32, tag="tpsum")
            for q in range(qn):
                pt = grp * TG + q
                qs = slice(q * P, (q + 1) * P)
                nc.tensor.transpose(ps0[:, qs], src_tile(pt)[:, 0:P], ident)
                nc.tensor.transpose(ps1[:, qs], src_tile(pt)[:, P:D], ident)
            cs = slice(grp * TG * P, grp * TG * P + qn * P)
            for dst, ps in ((dst0, ps0), (dst1, ps1)):
                if copy_rr[0] % 3 == 2:
                    nc.vector.tensor_copy(dst[:, cs], ps[:, 0 : qn * P])
                else:
                    nc.scalar.copy(dst[:, cs], ps[:, 0 : qn * P])
                copy_rr[0] += 1

        # ---- prep ----
        # Every DMA writes a FRESH destination (DMA descriptors carry at most
        # one sync wait; buffer reuse would need two).  The full point set is
        # loaded in chunk-sized DMAs, first chunk first so its transposes and
        # matmuls start while the rest streams in.
        tiles_per_chunk = col_chunk // P
        with tc.tile_pool(name="sqp", bufs=3) as sqp:
            sq_insts = []
            for rt in range(n_rtiles):
                nc.sync.dma_start(out=xrn[:, rt, :], in_=xr[rt])
                # per-row squared norm (f32): sum(x*x) along free dim
                sq_scr = sqp.tile([P, D], f32, tag="sqscr")
                sq_insts.append(nc.vector.scalar_tensor_tensor(
                    out=sq_scr,
                    in0=xrn[:, rt, :],
                    scalar=1.0,
                    in1=xrn[:, rt, :],
                    op0=ALU.mult,
                    op1=ALU.mult,
                    accum_out=sqr_all[:, rt : rt + 1],
                ))
            i_bias = nc.vector.tensor_scalar_mul(bias_all, sqr_all, -scale0)
            from concourse.tile import add_dep_helper
            # same-engine (DVE) ordering edge; no semaphore cost
            for si in sq_insts:
                add_dep_helper(i_bias.ins, si.ins, sync=False,
                               reason="row sq accum before bias")
            # sigb_all = sig_b - sig_scale*bias (folds the H-bias subtraction
            # into the final sigmoid's activation bias)
            nc.vector.tensor_scalar(
                out=sigb_all, in0=bias_all, scalar1=-sig_scale, scalar2=sig_b,
                op0=ALU.mult, op1=ALU.add,
            )
            for grp in range((n_rtiles + TG - 1) // TG):
                qn = min(TG, n_rtiles - grp * TG)
                transpose_group(lambda t: xrn[:, t, :], grp, qn, lhsT0, lhsT1)

        # ---- main loop: per column chunk, load + transpose its point tiles,
        # then sweep all row tiles; chunk c+1's loads/transposes overlap
        # chunk c's matmul/exp/reduce pipeline ----
        kpool = ctx.enter_context(tc.tile_pool(name="kpool", bufs=3))
        tpool = ctx.enter_context(tc.tile_pool(name="tpool", bufs=2))
        spool = ctx.enter_context(tc.tile_pool(name="stage", bufs=2))
        opool = ctx.enter_context(tc.tile_pool(name="outp", bufs=2))

        accum_insts = [[] for _ in range(n_rtiles)]

        def finalize_rt(rt):
            from concourse.tile import add_dep_helper

            bias_rt = bias_all[:, rt : rt + 1]
            Srow = spool.tile([P, 1], f32, tag="Srow")
            Urow = spool.tile([P, 1], f32, tag="Urow")
            red_s = nc.vector.tensor_reduce(Srow, S_all[rt], axis=AX.X, op=ALU.add)
            red_u = nc.vector.tensor_reduce(Urow, U_all[rt], axis=AX.X, op=ALU.add)
            # Ordering insurance for the strided accum writes: the reduces
            # run on DVE after every chunk's STT (same engine, no semaphore
            # cost); each STT is already semaphore-ordered after its EXP via
            # the kt tile, which transitively covers the ACT accum writes.
            for acc_inst in accum_insts[rt]:
                add_dep_helper(red_s.ins, acc_inst.ins, sync=False,
                               reason="S/U accum before row reduce")
                add_dep_helper(red_u.ins, acc_inst.ins, sync=False,
                               reason="S/U accum before row reduce")
            rS = spool.tile([P, 1], f32, tag="rS")
            nc.vector.reciprocal(rS, Srow)
            lnS = spool.tile([P, 1], f32, tag="lnS")
            nc.scalar.activation(out=lnS, in_=Srow, func=AF.Ln, bias=zero_b, scale=1.0)
            q1 = spool.tile([P, 1], f32, tag="q1")
            nc.vector.tensor_mul(q1, Urow, rS)
            # H = lnS - scale0*q1 - bias ; e1 = (-scale0)*q1 + lnS
            e1 = spool.tile([P, 1], f32, tag="e1")
            nc.vector.scalar_tensor_tensor(
                out=e1, in0=q1, scalar=-scale0, in1=lnS, op0=ALU.mult, op1=ALU.add
            )
            # control = 1/(1 + exp((H - target')/t)); the -bias term of H is
            # folded into sigb_all, so e1 feeds the Exp directly
            w = spool.tile([P, 1], f32, tag="w")
            nc.scalar.activation(
                out=w, in_=e1, func=AF.Exp,
                bias=sigb_all[:, rt : rt + 1], scale=sig_scale,
            )
            w1 = spool.tile([P, 1], f32, tag="w1")
            # ACT is stalled waiting on the last matmuls in this window while
            # DVE is congested with finalize work: run the +1 and the gating
            # multiply on ACT (Copy with float bias / per-partition AP scale)
            nc.scalar.activation(out=w1, in_=w, func=AF.Copy, bias=1.0, scale=1.0)
            nc.vector.reciprocal(ctrl_all[:, rt : rt + 1], w1)

            of_t = opool.tile([P, D], f32, tag="of")
            nc.scalar.activation(
                out=of_t, in_=xrn[:, rt, :], func=AF.Copy,
                bias=0.0, scale=ctrl_all[:, rt : rt + 1],
            )
            nc.sync.dma_start(out=of_re[rt], in_=of_t)
            octl = out_ctrl[rt * P : (rt + 1) * P].rearrange("(p one) -> p one", one=1)
            nc.sync.dma_start(out=octl, in_=ctrl_all[:, rt : rt + 1])

        for cc in range(n_chunks):
            for pt in range(cc * tiles_per_chunk, (cc + 1) * tiles_per_chunk):
                nc.sync.dma_start(out=XN[:, pt, :], in_=xf[pt])
            for grp in range(cc * tiles_per_chunk // TG,
                             (cc + 1) * tiles_per_chunk // TG):
                transpose_group(lambda t: XN[:, t, :], grp, TG, xT0, xT1)
            for rt in range(n_rtiles):
                rs = slice(rt * P, (rt + 1) * P)
                bias_rt = bias_all[:, rt : rt + 1]
                pm = psum_m.tile([P, col_chunk], f32, tag="mpsum")
                for g in range(n_groups):
                    c0 = cc * col_chunk + g * MM_N
                    gs = slice(g * MM_N, (g + 1) * MM_N)
                    nc.tensor.matmul(
                        pm[:, gs], lhsT0[:, rs], xT0[:, c0 : c0 + MM_N],
                        start=True, stop=False,
                    )
                for g in range(n_groups):
                    c0 = cc * col_chunk + g * MM_N
                    gs = slice(g * MM_N, (g + 1) * MM_N)
                    nc.tensor.matmul(
                        pm[:, gs], lhsT1[:, rs], xT1[:, c0 : c0 + MM_N],
                        start=False, stop=True,
                    )
                kt = kpool.tile([P, col_chunk], f32, tag="k")
                i_exp = nc.scalar.activation(
                    out=kt,
                    in_=pm,
                    func=AF.Exp,
                    bias=bias_rt,
                    scale=scale0,
                    accum_out=S_all[rt][:, cc : cc + 1],
                )
                # fused multiply + row-sum: U_part = sum_j k*m (out discarded)
                tt = tpool.tile([P, col_chunk], bf16, tag="tt")
                i_stt = nc.vector.scalar_tensor_tensor(
                    out=tt,
                    in0=pm,
                    scalar=1.0,
                    in1=kt,
                    op0=ALU.mult,
                    op1=ALU.mult,
                    accum_out=U_all[rt][:, cc : cc + 1],
                )
                accum_insts[rt].append(i_stt)
                if cc == n_chunks - 1:
                    # finalize this row tile while later ones still compute
                    finalize_rt(rt)

    _legalize_waits(nc)
    nc.finalize()
    return nc


def kernel(**inputs):
    feats = np.ascontiguousarray(np.asarray(inputs["features"], dtype=np.float32))
    t = float(np.asarray(inputs["temperature"]).reshape(-1)[0])
    target = float(np.asarray(inputs["target_entropy"]).reshape(-1)[0])
    B, S, D_ = feats.shape
    assert D_ == D
    N = B * S
    M = N // N_CORES
    x = feats.reshape(N, D_)

    nc = build_program(N, M, t, target)

    from concourse.bass_utils import run_bass_kernel_spmd

    ident_np = np.eye(P, dtype=np.float32)
    in_maps = [
        {
            "x_full": x,
            "ident_in": ident_np,
            "x_rows": np.ascontiguousarray(x[i * M : (i + 1) * M]),
        }
        for i in range(N_CORES)
    ]
    res = run_bass_kernel_spmd(nc, in_maps, list(range(N_CORES)), trace=TRACE)
    global LAST_RESULT
    LAST_RESULT = res
    feats_out = np.concatenate(
        [res.results[i]["out_feat"] for i in range(N_CORES)], axis=0
    ).reshape(B, S, D_)
    ctrl = np.concatenate(
        [res.results[i]["out_ctrl"] for i in range(N_CORES)], axis=0
    ).reshape(B, S)
    return feats_out.astype(np.float32), ctrl.astype(np.float32)


# revision 59
# speedup vs baseline: 1.0203x; 1.0203x over previous
"""Trainium2 Bass kernel for nn_EntropyController (retrieval_knn).

Math (reference, for x = features.reshape(N, D)):
    d2_ij   = ||x_i - x_j||^2
    k_ij    = exp(-d2_ij / (2 t^2))
    p_ij    = k_ij / S_i,  S_i = sum_j k_ij
    H_i     = -sum_j p_ij log(p_ij + 1e-6)
    control = sigmoid(-(H - target)/t);  out = features * control[..., None]

Kernel strategy (block-row data parallel over 8 cores, M = N/8 rows each):
    p_ij (and therefore H and the outputs) is invariant under any per-row
    shift of the exponent, so instead of -d2/(2t^2) the device evaluates
        e_ij = (dot(x_i, x_j) - sq_i) / t^2
    (shift by sq_j/2 - sq_i/2 per row): the -sq_i bias is a per-partition
    activation bias, which makes the whole exponent computable by a plain
    matmul -- no augmentation row for the column term.  For gaussian-like
    data e_ij <= ~12 off-diagonal (no f32 overflow) and e_ii = 0.

    The N x N map is never materialized.  Per [128, C] chunk:
      - TensorE: m = dot(x_i, x_j) into PSUM (two K=128 matmuls).
      - ScalarE: k = exp(scale*m + bias_i) PSUM->SBUF, with accum_out
        producing the row-sum S for free.
      - VectorE: one fused scalar_tensor_tensor gives U = sum_j k*m.
    Then sum_j k*e = scale*U + bias*S, so
        H = log S - (scale*U + bias*S)/S.
    The reference's +1e-6 inside the log shifts H by log1p(1e-6) per
    dominant mode; -log1p(1e-6) is folded into the sigmoid bias.
    sigmoid is computed as 1/(1+exp(z)) to stay in the exp/ln ACT table set.
"""

import math
from contextlib import ExitStack

import numpy as np

N_CORES = 8
D = 256
P = 128
MM_N = 512  # moving free dim per matmul (one PSUM bank of f32)


def _legalize_waits(nc, limit=1):
    """The deployed walrus accepts at most one sync-wait command per
    instruction; Tile's scheduler freely emits more.  Hoist the excess onto
    injected same-engine EventSemaphore instructions placed directly before
    the offender (engine programs are sequential, so the waits still gate
    it)."""
    import bass_rust
    from concourse import mybir

    n_new = 0
    for fn in nc.m.functions:
        for blk in fn.blocks:
            out = []
            for inst in blk.instructions:
                si = getattr(inst, "sync_info", None)
                waits = list(si.on_wait) if (si is not None and si.on_wait) else []
                if len(waits) > limit:
                    excess, keep = waits[:-limit], waits[-limit:]
                    for i, w in enumerate(excess):
                        ev = bass_rust.InstEventSemaphore(
                            name=f"{inst.name}-waitshim-{i}",
                            engine=inst.engine,
                            ins=[],
                            outs=[],
                        )
                        ev.sync_info = mybir.SyncInfo(on_wait=[w], on_update=[])
                        nc.inst_map[ev.name] = ev
                        out.append(ev)
                        n_new += 1
                    inst.sync_info = mybir.SyncInfo(
                        on_wait=keep, on_update=list(si.on_update)
                    )
                out.append(inst)
            blk.instructions = out
    return n_new


# Test-harness hooks (the grading path leaves these alone).
TRACE = False
LAST_RESULT = None


def build_program(n_pts, n_rows, temperature, target_entropy, col_chunk=1024):
    import concourse.bass as bass
    import concourse.tile as tile
    from concourse import mybir

    f32 = mybir.dt.float32
    bf16 = mybir.dt.bfloat16
    AF = mybir.ActivationFunctionType
    ALU = mybir.AluOpType
    AX = mybir.AxisListType

    assert n_pts % col_chunk == 0 and col_chunk % MM_N == 0
    assert n_rows % P == 0 and n_pts % P == 0
    n_ptiles = n_pts // P
    n_rtiles = n_rows // P
    n_chunks = n_pts // col_chunk
    n_groups = col_chunk // MM_N

    t = float(temperature)
    scale0 = 1.0 / (t * t)
    # z = (H - target - log1p(1e-6)) / t ; control = 1/(1+exp(z))
    sig_scale = 1.0 / t
    sig_bias = -(float(target_entropy) + math.log1p(1e-6)) / t

    nc = bass.Bass()
    x_full = nc.declare_dram_parameter("x_full", [n_pts, D], f32, isOutput=False)
    ident_in = nc.declare_dram_parameter("ident_in", [P, P], f32, isOutput=False)
    x_rows = nc.declare_dram_parameter("x_rows", [n_rows, D], f32, isOutput=False)
    out_feat = nc.declare_dram_parameter("out_feat", [n_rows, D], f32, isOutput=True)
    out_ctrl = nc.declare_dram_parameter("out_ctrl", [n_rows], f32, isOutput=True)

    xf = x_full[:, :].rearrange("(t p) d -> t p d", p=P)
    xr = x_rows[:, :].rearrange("(t p) d -> t p d", p=P)
    of_re = out_feat[:, :].rearrange("(t p) d -> t p d", p=P)

    with tile.TileContext(nc) as tc, ExitStack() as ctx:
        persist = ctx.enter_context(tc.tile_pool(name="persist", bufs=1))
        # 3x [128,1024] f32 accumulators (6 banks) + 2x [128,512] transpose
        # tiles (2 banks) = all 8 PSUM banks
        psum_m = ctx.enter_context(tc.tile_pool(name="psum_m", bufs=3, space="PSUM"))
        psum_t = ctx.enter_context(tc.tile_pool(name="psum_t", bufs=2, space="PSUM"))

        xT0 = persist.tile([P, n_pts], bf16)  # dims 0..127, all points
        xT1 = persist.tile([P, n_pts], bf16)  # dims 128..255
        lhsT0 = persist.tile([P, n_rows], bf16)
        lhsT1 = persist.tile([P, n_rows], bf16)
        xrn = persist.tile([P, n_rtiles, D], f32)  # this core's rows, natural
        sqr_all = persist.tile([P, n_rtiles], f32)  # per-row |x|^2 (f32)
        bias_all = persist.tile([P, n_rtiles], f32)  # -sq_i/t^2
        ctrl_all = persist.tile([P, n_rtiles], f32)
        S_all = [persist.tile([P, n_chunks], f32, name=f"S_acc_{r}", tag=f"S_acc_{r}")
                 for r in range(n_rtiles)]
        U_all = [persist.tile([P, n_chunks], f32, name=f"U_acc_{r}", tag=f"U_acc_{r}")
                 for r in range(n_rtiles)]
        zero_b = persist.tile([P, 1], f32)
        sig_b = persist.tile([P, 1], f32)
        ident = persist.tile([P, P], f32)  # for PE-mode transposes (f32 in/out)
        XN = persist.tile([P, n_ptiles, D], f32)  # full point set, natural

        sigb_all = persist.tile([P, n_rtiles], f32)  # sig_b - sig_scale*bias_i

        nc.vector.memset(zero_b, 0.0)
        nc.vector.memset(sig_b, sig_bias)
        # identity comes from DRAM: removes the gpsimd memset+affine_select
        # from the transpose critical path
        nc.sync.dma_start(out=ident, in_=ident_in[:, :])

        # PE-mode f32 128x128 transposes, 4 tiles batched per PSUM bank tile;
        # the ACT PSUM->SBUF copy performs the f32->bf16 cast.  Transposes get
        # their own PSUM tag so they pipeline independently of the matmul
        # accumulators.
        TG = 4  # point-tiles per transpose group

        copy_rr = [0]  # round-robin PSUM->SBUF copies: 2 on ACT, 1 on DVE

        def transpose_group(src_tile, grp, qn, dst0, dst1, pool=None):
            pool = pool if pool is not None else psum_t
            tag = "mpsum" if pool is psum_m else "tpsum"
            ps0 = pool.tile([P, TG * P], f32, tag=tag)
            ps1 = pool.tile([P, TG * P], f32, tag=tag)
            for q in range(qn):
                pt = grp * TG + q
                qs = slice(q * P, (q + 1) * P)
                nc.tensor.transpose(ps0[:, qs], src_tile(pt)[:, 0:P], ident)
                nc.tensor.transpose(ps1[:, qs], src_tile(pt)[:, P:D], ident)
            cs = slice(grp * TG * P, grp * TG * P + qn * P)
            for dst, ps in ((dst0, ps0), (dst1, ps1)):
                if copy_rr[0] % 3 == 2:
                    nc.vector.tensor_copy(dst[:, cs], ps[:, 0 : qn * P])
                else:
                    nc.scalar.copy(dst[:, cs], ps[:, 0 : qn * P])
                copy_rr[0] += 1

        # ---- prep ----
        # Every DMA writes a FRESH destination (DMA descriptors carry at most
        # one sync wait; buffer reuse would need two).  The full point set is
        # loaded in chunk-sized DMAs, first chunk first so its transposes and
        # matmuls start while the rest streams in.
        tiles_per_chunk = col_chunk // P
        with tc.tile_pool(name="sqp", bufs=3) as sqp:
            sq_insts = []
            for rt in range(n_rtiles):
                nc.sync.dma_start(out=xrn[:, rt, :], in_=xr[rt])
                # per-row squared norm (f32): sum(x*x) along free dim
                sq_scr = sqp.tile([P, D], f32, tag="sqscr")
                sq_insts.append(nc.vector.scalar_tensor_tensor(
                    out=sq_scr,
                    in0=xrn[:, rt, :],
                    scalar=1.0,
                    in1=xrn[:, rt, :],
                    op0=ALU.mult,
                    op1=ALU.mult,
                    accum_out=sqr_all[:, rt : rt + 1],
                ))
            i_bias = nc.vector.tensor_scalar_mul(bias_all, sqr_all, -scale0)
            from concourse.tile import add_dep_helper
            # same-engine (DVE) ordering edge; no semaphore cost
            for si in sq_insts:
                add_dep_helper(i_bias.ins, si.ins, sync=False,
                               reason="row sq accum before bias")
            # sigb_all = sig_b - sig_scale*bias (folds the H-bias subtraction
            # into the final sigmoid's activation bias)
            nc.vector.tensor_scalar(
                out=sigb_all, in0=bias_all, scalar1=-sig_scale, scalar2=sig_b,
                op0=ALU.mult, op1=ALU.add,
            )
            # rows borrow the (still idle) matmul PSUM slots so the row and
            # chunk-0 transpose paths use disjoint slots and interleave
            for grp in range((n_rtiles + TG - 1) // TG):
                qn = min(TG, n_rtiles - grp * TG)
                transpose_group(lambda t: xrn[:, t, :], grp, qn, lhsT0, lhsT1,
                                pool=psum_m)

        # ---- main loop: per column chunk, load + transpose its point tiles,
        # then sweep all row tiles; chunk c+1's loads/transposes overlap
        # chunk c's matmul/exp/reduce pipeline ----
        kpool = ctx.enter_context(tc.tile_pool(name="kpool", bufs=3))
        tpool = ctx.enter_context(tc.tile_pool(name="tpool", bufs=2))
        spool = ctx.enter_context(tc.tile_pool(name="stage", bufs=2))
        opool = ctx.enter_context(tc.tile_pool(name="outp", bufs=2))

        accum_insts = [[] for _ in range(n_rtiles)]

        def finalize_rt(rt):
            from concourse.tile import add_dep_helper

            bias_rt = bias_all[:, rt : rt + 1]
            Srow = spool.tile([P, 1], f32, tag="Srow")
            Urow = spool.tile([P, 1], f32, tag="Urow")
            red_s = nc.vector.tensor_reduce(Srow, S_all[rt], axis=AX.X, op=ALU.add)
            red_u = nc.vector.tensor_reduce(Urow, U_all[rt], axis=AX.X, op=ALU.add)
            # Ordering insurance for the strided accum writes: the reduces
            # run on DVE after every chunk's STT (same engine, no semaphore
            # cost); each STT is already semaphore-ordered after its EXP via
            # the kt tile, which transitively covers the ACT accum writes.
            for acc_inst in accum_insts[rt]:
                add_dep_helper(red_s.ins, acc_inst.ins, sync=False,
                               reason="S/U accum before row reduce")
                add_dep_helper(red_u.ins, acc_inst.ins, sync=False,
                               reason="S/U accum before row reduce")
            rS = spool.tile([P, 1], f32, tag="rS")
            nc.vector.reciprocal(rS, Srow)
            lnS = spool.tile([P, 1], f32, tag="lnS")
            nc.scalar.activation(out=lnS, in_=Srow, func=AF.Ln, bias=zero_b, scale=1.0)
            q1 = spool.tile([P, 1], f32, tag="q1")
            nc.vector.tensor_mul(q1, Urow, rS)
            # H = lnS - scale0*q1 - bias ; e1 = (-scale0)*q1 + lnS
            e1 = spool.tile([P, 1], f32, tag="e1")
            nc.vector.scalar_tensor_tensor(
                out=e1, in0=q1, scalar=-scale0, in1=lnS, op0=ALU.mult, op1=ALU.add
            )
            # control = 1/(1 + exp((H - target')/t)); the -bias term of H is
            # folded into sigb_all, so e1 feeds the Exp directly
            w = spool.tile([P, 1], f32, tag="w")
            nc.scalar.activation(
                out=w, in_=e1, func=AF.Exp,
                bias=sigb_all[:, rt : rt + 1], scale=sig_scale,
            )
            w1 = spool.tile([P, 1], f32, tag="w1")
            # ACT is stalled waiting on the last matmuls in this window while
            # DVE is congested with finalize work: run the +1 and the gating
            # multiply on ACT (Copy with float bias / per-partition AP scale)
            nc.scalar.activation(out=w1, in_=w, func=AF.Copy, bias=1.0, scale=1.0)
            nc.vector.reciprocal(ctrl_all[:, rt : rt + 1], w1)

            of_t = opool.tile([P, D], f32, tag="of")
            nc.scalar.activation(
                out=of_t, in_=xrn[:, rt, :], func=AF.Copy,
                bias=0.0, scale=ctrl_all[:, rt : rt + 1],
            )
            nc.sync.dma_start(out=of_re[rt], in_=of_t)
            octl = out_ctrl[rt * P : (rt + 1) * P].rearrange("(p one) -> p one", one=1)
            nc.sync.dma_start(out=octl, in_=ctrl_all[:, rt : rt + 1])

        for cc in range(n_chunks):
            for pt in range(cc * tiles_per_chunk, (cc + 1) * tiles_per_chunk):
                nc.sync.dma_start(out=XN[:, pt, :], in_=xf[pt])
            for grp in range(cc * tiles_per_chunk // TG,
                             (cc + 1) * tiles_per_chunk // TG):
                transpose_group(lambda t: XN[:, t, :], grp, TG, xT0, xT1)
            for rt in range(n_rtiles):
                rs = slice(rt * P, (rt + 1) * P)
                bias_rt = bias_all[:, rt : rt + 1]
                pm = psum_m.tile([P, col_chunk], f32, tag="mpsum")
                for g in range(n_groups):
                    c0 = cc * col_chunk + g * MM_N
                    gs = slice(g * MM_N, (g + 1) * MM_N)
                    nc.tensor.matmul(
                        pm[:, gs], lhsT0[:, rs], xT0[:, c0 : c0 + MM_N],
                        start=True, stop=False,
                    )
                for g in range(n_groups):
                    c0 = cc * col_chunk + g * MM_N
                    gs = slice(g * MM_N, (g + 1) * MM_N)
                    nc.tensor.matmul(
                        pm[:, gs], lhsT1[:, rs], xT1[:, c0 : c0 + MM_N],
                        start=False, stop=True,
                    )
                kt = kpool.tile([P, col_chunk], f32, tag="k")
                i_exp = nc.scalar.activation(
                    out=kt,
                    in_=pm,
                    func=AF.Exp,
                    bias=bias_rt,
                    scale=scale0,
                    accum_out=S_all[rt][:, cc : cc + 1],
                )
                # fused multiply + row-sum: U_part = sum_j k*m (out discarded)
                tt = tpool.tile([P, col_chunk], bf16, tag="tt")
                i_stt = nc.vector.scalar_tensor_tensor(
                    out=tt,
                    in0=pm,
                    scalar=1.0,
                    in1=kt,
                    op0=ALU.mult,
                    op1=ALU.mult,
                    accum_out=U_all[rt][:, cc : cc + 1],
                )
                accum_insts[rt].append(i_stt)
                if cc == n_chunks - 1:
                    # finalize this row tile while later ones still compute
                    finalize_rt(rt)

    _legalize_waits(nc)
    nc.finalize()
    return nc


def kernel(**inputs):
    feats = np.ascontiguousarray(np.asarray(inputs["features"], dtype=np.float32))
    t = float(np.asarray(inputs["temperature"]).reshape(-1)[0])
    target = float(np.asarray(inputs["target_entropy"]).reshape(-1)[0])
    B, S, D_ = feats.shape
    assert D_ == D
    N = B * S
    M = N // N_CORES
    x = feats.reshape(N, D_)

    nc = build_program(N, M, t, target)

    from concourse.bass_utils import run_bass_kernel_spmd

    ident_np = np.eye(P, dtype=np.float32)
    in_maps = [
        {
            "x_full": x,
            "ident_in": ident_np,
            "x_rows": np.ascontiguousarray(x[i * M : (i + 1) * M]),
        }
        for i in range(N_CORES)
    ]
    res = run_bass_kernel_spmd(nc, in_maps, list(range(N_CORES)), trace=TRACE)
    global LAST_RESULT
    LAST_RESULT = res
    feats_out = np.concatenate(
        [res.results[i]["out_feat"] for i in range(N_CORES)], axis=0
    ).reshape(B, S, D_)
    ctrl = np.concatenate(
        [res.results[i]["out_ctrl"] for i in range(N_CORES)], axis=0
    ).reshape(B, S)
    return feats_out.astype(np.float32), ctrl.astype(np.float32)


# revision 62
# speedup vs baseline: 1.0272x; 1.0068x over previous
"""Trainium2 Bass kernel for nn_EntropyController (retrieval_knn).

Math (reference, for x = features.reshape(N, D)):
    d2_ij   = ||x_i - x_j||^2
    k_ij    = exp(-d2_ij / (2 t^2))
    p_ij    = k_ij / S_i,  S_i = sum_j k_ij
    H_i     = -sum_j p_ij log(p_ij + 1e-6)
    control = sigmoid(-(H - target)/t);  out = features * control[..., None]

Kernel strategy (block-row data parallel over 8 cores, M = N/8 rows each):
    p_ij (and therefore H and the outputs) is invariant under any per-row
    shift of the exponent, so instead of -d2/(2t^2) the device evaluates
        e_ij = (dot(x_i, x_j) - sq_i) / t^2
    (shift by sq_j/2 - sq_i/2 per row): the -sq_i bias is a per-partition
    activation bias, which makes the whole exponent computable by a plain
    matmul -- no augmentation row for the column term.  For gaussian-like
    data e_ij <= ~12 off-diagonal (no f32 overflow) and e_ii = 0.

    The N x N map is never materialized.  Per [128, C] chunk:
      - TensorE: m = dot(x_i, x_j) into PSUM (two K=128 matmuls).
      - ScalarE: k = exp(scale*m + bias_i) PSUM->SBUF, with accum_out
        producing the row-sum S for free.
      - VectorE: one fused scalar_tensor_tensor gives U = sum_j k*m.
    Then sum_j k*e = scale*U + bias*S, so
        H = log S - (scale*U + bias*S)/S.
    The reference's +1e-6 inside the log shifts H by log1p(1e-6) per
    dominant mode; -log1p(1e-6) is folded into the sigmoid bias.
    sigmoid is computed as 1/(1+exp(z)) to stay in the exp/ln ACT table set.
"""

import math
from contextlib import ExitStack

import numpy as np

N_CORES = 8
D = 256
P = 128
MM_N = 512  # moving free dim per matmul (one PSUM bank of f32)


def _legalize_waits(nc, limit=1):
    """The deployed walrus accepts at most one sync-wait command per
    instruction; Tile's scheduler freely emits more.  Hoist the excess onto
    injected same-engine EventSemaphore instructions placed directly before
    the offender (engine programs are sequential, so the waits still gate
    it)."""
    import bass_rust
    from concourse import mybir

    n_new = 0
    for fn in nc.m.functions:
        for blk in fn.blocks:
            out = []
            for inst in blk.instructions:
                si = getattr(inst, "sync_info", None)
                waits = list(si.on_wait) if (si is not None and si.on_wait) else []
                if len(waits) > limit:
                    excess, keep = waits[:-limit], waits[-limit:]
                    for i, w in enumerate(excess):
                        ev = bass_rust.InstEventSemaphore(
                            name=f"{inst.name}-waitshim-{i}",
                            engine=inst.engine,
                            ins=[],
                            outs=[],
                        )
                        ev.sync_info = mybir.SyncInfo(on_wait=[w], on_update=[])
                        nc.inst_map[ev.name] = ev
                        out.append(ev)
                        n_new += 1
                    inst.sync_info = mybir.SyncInfo(
                        on_wait=keep, on_update=list(si.on_update)
                    )
                out.append(inst)
            blk.instructions = out
    return n_new


# Test-harness hooks (the grading path leaves these alone).
TRACE = False
LAST_RESULT = None


def build_program(n_pts, n_rows, temperature, target_entropy, col_chunk=1024):
    import concourse.bass as bass
    import concourse.tile as tile
    from concourse import mybir

    f32 = mybir.dt.float32
    bf16 = mybir.dt.bfloat16
    AF = mybir.ActivationFunctionType
    ALU = mybir.AluOpType
    AX = mybir.AxisListType

    assert n_pts % col_chunk == 0 and col_chunk % MM_N == 0
    assert n_rows % P == 0 and n_pts % P == 0
    n_ptiles = n_pts // P
    n_rtiles = n_rows // P
    n_chunks = n_pts // col_chunk
    n_groups = col_chunk // MM_N

    t = float(temperature)
    scale0 = 1.0 / (t * t)
    # z = (H - target - log1p(1e-6)) / t ; control = 1/(1+exp(z))
    sig_scale = 1.0 / t
    sig_bias = -(float(target_entropy) + math.log1p(1e-6)) / t

    nc = bass.Bass()
    x_full = nc.declare_dram_parameter("x_full", [n_pts, D], f32, isOutput=False)
    ident_in = nc.declare_dram_parameter("ident_in", [P, P], f32, isOutput=False)
    x_rows = nc.declare_dram_parameter("x_rows", [n_rows, D], f32, isOutput=False)
    out_feat = nc.declare_dram_parameter("out_feat", [n_rows, D], f32, isOutput=True)
    out_ctrl = nc.declare_dram_parameter("out_ctrl", [n_rows], f32, isOutput=True)

    xf = x_full[:, :].rearrange("(t p) d -> t p d", p=P)
    xr = x_rows[:, :].rearrange("(t p) d -> t p d", p=P)
    of_re = out_feat[:, :].rearrange("(t p) d -> t p d", p=P)

    with tile.TileContext(nc) as tc, ExitStack() as ctx:
        persist = ctx.enter_context(tc.tile_pool(name="persist", bufs=1))
        # 3x [128,1024] f32 accumulators (6 banks) + 2x [128,512] transpose
        # tiles (2 banks) = all 8 PSUM banks
        psum_m = ctx.enter_context(tc.tile_pool(name="psum_m", bufs=3, space="PSUM"))
        psum_t = ctx.enter_context(tc.tile_pool(name="psum_t", bufs=2, space="PSUM"))

        xT0 = persist.tile([P, n_pts], bf16)  # dims 0..127, all points
        xT1 = persist.tile([P, n_pts], bf16)  # dims 128..255
        lhsT0 = persist.tile([P, n_rows], bf16)
        lhsT1 = persist.tile([P, n_rows], bf16)
        xrn = persist.tile([P, n_rtiles, D], f32)  # this core's rows, natural
        sqr_all = persist.tile([P, n_rtiles], f32)  # per-row |x|^2 (f32)
        bias_all = persist.tile([P, n_rtiles], f32)  # -sq_i/t^2
        ctrl_all = persist.tile([P, n_rtiles], f32)
        S_all = [persist.tile([P, n_chunks], f32, name=f"S_acc_{r}", tag=f"S_acc_{r}")
                 for r in range(n_rtiles)]
        U_all = [persist.tile([P, n_chunks], f32, name=f"U_acc_{r}", tag=f"U_acc_{r}")
                 for r in range(n_rtiles)]
        zero_b = persist.tile([P, 1], f32)
        sig_b = persist.tile([P, 1], f32)
        ident = persist.tile([P, P], f32)  # for PE-mode transposes (f32 in/out)
        XN = persist.tile([P, n_ptiles, D], f32)  # full point set, natural

        sigb_all = persist.tile([P, n_rtiles], f32)  # sig_b - sig_scale*bias_i

        nc.vector.memset(zero_b, 0.0)
        nc.vector.memset(sig_b, sig_bias)
        # identity comes from DRAM: removes the gpsimd memset+affine_select
        # from the transpose critical path
        nc.sync.dma_start(out=ident, in_=ident_in[:, :])

        # PE-mode f32 128x128 transposes, 4 tiles batched per PSUM bank tile;
        # the ACT PSUM->SBUF copy performs the f32->bf16 cast.  Transposes get
        # their own PSUM tag so they pipeline independently of the matmul
        # accumulators.
        TG = 4  # point-tiles per transpose group

        copy_rr = [0]  # round-robin PSUM->SBUF copies: 2 on ACT, 1 on DVE

        def transpose_group(src_tile, grp, qn, dst0, dst1, pool=None):
            pool = pool if pool is not None else psum_t
            tag = "mpsum" if pool is psum_m else "tpsum"
            ps0 = pool.tile([P, TG * P], f32, tag=tag)
            ps1 = pool.tile([P, TG * P], f32, tag=tag)
            for q in range(qn):
                pt = grp * TG + q
                qs = slice(q * P, (q + 1) * P)
                nc.tensor.transpose(ps0[:, qs], src_tile(pt)[:, 0:P], ident)
                nc.tensor.transpose(ps1[:, qs], src_tile(pt)[:, P:D], ident)
            cs = slice(grp * TG * P, grp * TG * P + qn * P)
            for dst, ps in ((dst0, ps0), (dst1, ps1)):
                if copy_rr[0] % 3 == 2:
                    nc.vector.tensor_copy(dst[:, cs], ps[:, 0 : qn * P])
                else:
                    nc.scalar.copy(dst[:, cs], ps[:, 0 : qn * P])
                copy_rr[0] += 1

        # ---- prep ----
        # Every DMA writes a FRESH destination (DMA descriptors carry at most
        # one sync wait; buffer reuse would need two).  The full point set is
        # loaded in chunk-sized DMAs, first chunk first so its transposes and
        # matmuls start while the rest streams in.
        tiles_per_chunk = col_chunk // P
        with tc.tile_pool(name="sqp", bufs=3) as sqp:
            sq_insts = []
            for rt in range(n_rtiles):
                nc.sync.dma_start(out=xrn[:, rt, :], in_=xr[rt])
                # per-row squared norm (f32): sum(x*x) along free dim
                sq_scr = sqp.tile([P, D], f32, tag="sqscr")
                sq_insts.append(nc.vector.scalar_tensor_tensor(
                    out=sq_scr,
                    in0=xrn[:, rt, :],
                    scalar=1.0,
                    in1=xrn[:, rt, :],
                    op0=ALU.mult,
                    op1=ALU.mult,
                    accum_out=sqr_all[:, rt : rt + 1],
                ))
            i_bias = nc.vector.tensor_scalar_mul(bias_all, sqr_all, -scale0)
            from concourse.tile import add_dep_helper
            # same-engine (DVE) ordering edge; no semaphore cost
            for si in sq_insts:
                add_dep_helper(i_bias.ins, si.ins, sync=False,
                               reason="row sq accum before bias")
            # sigb_all = sig_b - sig_scale*bias (folds the H-bias subtraction
            # into the final sigmoid's activation bias)
            nc.vector.tensor_scalar(
                out=sigb_all, in0=bias_all, scalar1=-sig_scale, scalar2=sig_b,
                op0=ALU.mult, op1=ALU.add,
            )
            # rows borrow the (still idle) matmul PSUM slots so the row and
            # chunk-0 transpose paths use disjoint slots and interleave
            for grp in range((n_rtiles + TG - 1) // TG):
                qn = min(TG, n_rtiles - grp * TG)
                transpose_group(lambda t: xrn[:, t, :], grp, qn, lhsT0, lhsT1,
                                pool=psum_m)

        # ---- main loop: per column chunk, load + transpose its point tiles,
        # then sweep all row tiles; chunk c+1's loads/transposes overlap
        # chunk c's matmul/exp/reduce pipeline ----
        kpool = ctx.enter_context(tc.tile_pool(name="kpool", bufs=3))
        tpool = ctx.enter_context(tc.tile_pool(name="tpool", bufs=2))
        spool = ctx.enter_context(tc.tile_pool(name="stage", bufs=4))
        opool = ctx.enter_context(tc.tile_pool(name="outp", bufs=4))

        accum_insts = [[] for _ in range(n_rtiles)]

        def finalize_rt(rt):
            from concourse.tile import add_dep_helper

            bias_rt = bias_all[:, rt : rt + 1]
            Srow = spool.tile([P, 1], f32, tag="Srow")
            Urow = spool.tile([P, 1], f32, tag="Urow")
            red_s = nc.vector.tensor_reduce(Srow, S_all[rt], axis=AX.X, op=ALU.add)
            red_u = nc.vector.tensor_reduce(Urow, U_all[rt], axis=AX.X, op=ALU.add)
            # Ordering insurance for the strided accum writes: the reduces
            # run on DVE after every chunk's STT (same engine, no semaphore
            # cost); each STT is already semaphore-ordered after its EXP via
            # the kt tile, which transitively covers the ACT accum writes.
            for acc_inst in accum_insts[rt]:
                add_dep_helper(red_s.ins, acc_inst.ins, sync=False,
                               reason="S/U accum before row reduce")
                add_dep_helper(red_u.ins, acc_inst.ins, sync=False,
                               reason="S/U accum before row reduce")
            rS = spool.tile([P, 1], f32, tag="rS")
            nc.vector.reciprocal(rS, Srow)
            lnS = spool.tile([P, 1], f32, tag="lnS")
            nc.scalar.activation(out=lnS, in_=Srow, func=AF.Ln, bias=zero_b, scale=1.0)
            q1 = spool.tile([P, 1], f32, tag="q1")
            nc.vector.tensor_mul(q1, Urow, rS)
            # H = lnS - scale0*q1 - bias ; e1 = (-scale0)*q1 + lnS
            e1 = spool.tile([P, 1], f32, tag="e1")
            nc.vector.scalar_tensor_tensor(
                out=e1, in0=q1, scalar=-scale0, in1=lnS, op0=ALU.mult, op1=ALU.add
            )
            # control = 1/(1 + exp((H - target')/t)); the -bias term of H is
            # folded into sigb_all, so e1 feeds the Exp directly
            w = spool.tile([P, 1], f32, tag="w")
            nc.scalar.activation(
                out=w, in_=e1, func=AF.Exp,
                bias=sigb_all[:, rt : rt + 1], scale=sig_scale,
            )
            w1 = spool.tile([P, 1], f32, tag="w1")
            # ACT is stalled waiting on the last matmuls in this window while
            # DVE is congested with finalize work: run the +1 and the gating
            # multiply on ACT (Copy with float bias / per-partition AP scale)
            nc.scalar.activation(out=w1, in_=w, func=AF.Copy, bias=1.0, scale=1.0)
            nc.vector.reciprocal(ctrl_all[:, rt : rt + 1], w1)

            of_t = opool.tile([P, D], f32, tag="of")
            nc.scalar.activation(
                out=of_t, in_=xrn[:, rt, :], func=AF.Copy,
                bias=0.0, scale=ctrl_all[:, rt : rt + 1],
            )
            nc.sync.dma_start(out=of_re[rt], in_=of_t)
            octl = out_ctrl[rt * P : (rt + 1) * P].rearrange("(p one) -> p one", one=1)
            nc.sync.dma_start(out=octl, in_=ctrl_all[:, rt : rt + 1])

        for cc in range(n_chunks):
            for pt in range(cc * tiles_per_chunk, (cc + 1) * tiles_per_chunk):
                nc.sync.dma_start(out=XN[:, pt, :], in_=xf[pt])
            for grp in range(cc * tiles_per_chunk // TG,
                             (cc + 1) * tiles_per_chunk // TG):
                transpose_group(lambda t: XN[:, t, :], grp, TG, xT0, xT1)
            for rt in range(n_rtiles):
                rs = slice(rt * P, (rt + 1) * P)
                bias_rt = bias_all[:, rt : rt + 1]
                pm = psum_m.tile([P, col_chunk], f32, tag="mpsum")
                for g in range(n_groups):
                    c0 = cc * col_chunk + g * MM_N
                    gs = slice(g * MM_N, (g + 1) * MM_N)
                    nc.tensor.matmul(
                        pm[:, gs], lhsT0[:, rs], xT0[:, c0 : c0 + MM_N],
                        start=True, stop=False,
                    )
                for g in range(n_groups):
                    c0 = cc * col_chunk + g * MM_N
                    gs = slice(g * MM_N, (g + 1) * MM_N)
                    nc.tensor.matmul(
                        pm[:, gs], lhsT1[:, rs], xT1[:, c0 : c0 + MM_N],
                        start=False, stop=True,
                    )
                kt = kpool.tile([P, col_chunk], f32, tag="k")
                i_exp = nc.scalar.activation(
                    out=kt,
                    in_=pm,
                    func=AF.Exp,
                    bias=bias_rt,
                    scale=scale0,
                    accum_out=S_all[rt][:, cc : cc + 1],
                )
                # fused multiply + row-sum: U_part = sum_j k*m (out discarded)
                tt = tpool.tile([P, col_chunk], bf16, tag="tt")
                i_stt = nc.vector.scalar_tensor_tensor(
                    out=tt,
                    in0=pm,
                    scalar=1.0,
                    in1=kt,
                    op0=ALU.mult,
                    op1=ALU.mult,
                    accum_out=U_all[rt][:, cc : cc + 1],
                )
                accum_insts[rt].append(i_stt)
                if cc == n_chunks - 1:
                    # finalize this row tile while later ones still compute
                    finalize_rt(rt)

    _legalize_waits(nc)
    nc.finalize()
    return nc


def kernel(**inputs):
    feats = np.ascontiguousarray(np.asarray(inputs["features"], dtype=np.float32))
    t = float(np.asarray(inputs["temperature"]).reshape(-1)[0])
    target = float(np.asarray(inputs["target_entropy"]).reshape(-1)[0])
    B, S, D_ = feats.shape
    assert D_ == D
    N = B * S
    M = N // N_CORES
    x = feats.reshape(N, D_)

    nc = build_program(N, M, t, target)

    from concourse.bass_utils import run_bass_kernel_spmd

    ident_np = np.eye(P, dtype=np.float32)
    in_maps = [
        {
            "x_full": x,
            "ident_in": ident_np,
            "x_rows": np.ascontiguousarray(x[i * M : (i + 1) * M]),
        }
        for i in range(N_CORES)
    ]
    res = run_bass_kernel_spmd(nc, in_maps, list(range(N_CORES)), trace=TRACE)
    global LAST_RESULT
    LAST_RESULT = res
    feats_out = np.concatenate(
        [res.results[i]["out_feat"] for i in range(N_CORES)], axis=0
    ).reshape(B, S, D_)
    ctrl = np.concatenate(
        [res.results[i]["out_ctrl"] for i in range(N_CORES)], axis=0
    ).reshape(B, S)
    return feats_out.astype(np.float32), ctrl.astype(np.float32)
